# revision 12
# baseline (speedup 1.0000x reference)
"""Trainium2 Bass kernel for nn_Decoder_66065186947373 (3-stage decoder with
patch-expand + skip-concat + neighborhood attention, K=8 window).

Sharding: data-parallel over batch B=8 across 8 NeuronCores; params replicated.

Layout strategy: activations live on-chip transposed ([channels, tokens]) so
every linear is a single lhsT=W matmul; V is produced in natural layout on a
-4-shifted token grid so the banded attention P@V needs only 2 matmuls per
128-query block. All matmuls run in float32r (11-bit mantissa, full fp32 PSUM
accumulation). Window clipping (duplicated boundary keys) and rel_bias are
folded into host-built additive mask tiles via log-sum-exp.

Host-side weight folding:
  We0 = W_exp[:, :d] @ W_skip[:d]   (even tokens of patch-expand + skip)
  We1 = W_exp[:, d:] @ W_skip[:d]   (odd tokens)
  Wb  = W_skip[d:]                  (encoder skip half)
  Wvo = Wv @ Wo                     (V and output projection fused; exact by
                                     associativity since softmax rows sum to 1)
  bo' = bv @ Wo + bo is folded into the next stage's skip bias; the last
  stage applies it at the output evacuation.
"""

import sys

if "/opt/trn_rl_repo" not in sys.path:
    sys.path.insert(0, "/opt/trn_rl_repo")

import numpy as np

import concourse.bacc as bacc
import concourse.mybir as mybir
from concourse import tile
from concourse.masks import make_identity

F32 = mybir.dt.float32
F32R = mybir.dt.float32r

N_CORES = 8
K_NEIGH = 8
B, N0, C0 = 8, 1024, 1024  # x: (8, 1024, 1024)

STAGES = [
    dict(Cin=1024, d=512, N=2048),
    dict(Cin=512, d=256, N=4096),
    dict(Cin=256, d=128, N=8192),
]


# ---------------------------------------------------------------- kernel build
def build_decoder(n_iters=1):
    nc = bacc.Bacc("TRN2", target_bir_lowering=False, debug=False,
                   num_devices=N_CORES)

    inp = {}

    def din(name, shape, dt=F32R):
        inp[name] = nc.dram_tensor(name, list(shape), dt, kind="ExternalInput")
        return inp[name]

    din("xT", (1024, 1024))
    for s, st in enumerate(STAGES):
        Cin, d, N = st["Cin"], st["d"], st["N"]
        din(f"encE{s}", (d, N // 2))
        din(f"encO{s}", (d, N // 2))
        din(f"We0_{s}", (Cin, d))
        din(f"We1_{s}", (Cin, d))
        din(f"Wb_{s}", (d, d))
        din(f"Wq_{s}", (d, d))
        din(f"Wk_{s}", (d, d))
        din(f"Wvo_{s}", (d, d))
        din(f"bskipE_{s}", (128, d // 128), F32)
        din(f"bskipO_{s}", (128, d // 128), F32)
        din(f"bq_{s}", (128, d // 128), F32)
        din(f"bk_{s}", (128, d // 128), F32)
        din(f"mask0_{s}", (128, 136), F32)
        din(f"maskm_{s}", (128, 136), F32)
        din(f"maskl_{s}", (128, 132), F32)
    din("bo_2", (128, 1), F32)
    out = nc.dram_tensor("out", [128, 8192], F32, kind="ExternalOutput")

    with tile.TileContext(nc) as tc:
        if n_iters > 1:
            with tc.For_i(0, n_iters, 1):
                _emit(nc, tc, inp, out)
        else:
            _emit(nc, tc, inp, out)
    nc.compile()
    return nc


def _emit(nc, tc, inp, out):
    from contextlib import ExitStack

    with ExitStack() as ctx:
        const = ctx.enter_context(tc.tile_pool(name="const", bufs=1))
        ident = const.tile([128, 128], F32)
        make_identity(nc, ident[:])

        # one shared PSUM pool; tags keep total within the 8 banks
        psum = ctx.enter_context(tc.tile_pool(name="psum", bufs=1, space="PSUM"))

        # per-stage small constants
        stc = {}
        for s, st in enumerate(STAGES):
            d = st["d"]
            t = {}
            for nm in ("bskipE", "bskipO", "bq", "bk"):
                t[nm] = const.tile([128, d // 128], F32, name=f"{nm}{s}")
                nc.sync.dma_start(out=t[nm][:], in_=inp[f"{nm}_{s}"][:])
            for nm in ("mask0", "maskm"):
                t[nm] = const.tile([128, 136], F32, name=f"{nm}{s}")
                nc.sync.dma_start(out=t[nm][:], in_=inp[f"{nm}_{s}"][:])
            t["maskl"] = const.tile([128, 132], F32, name=f"maskl{s}")
            nc.sync.dma_start(out=t["maskl"][:], in_=inp[f"maskl_{s}"][:])
            stc[s] = t
        bo2 = const.tile([128, 1], F32)
        nc.sync.dma_start(out=bo2[:], in_=inp["bo_2"][:])

        in_cm = tc.tile_pool(name="act0", bufs=1, side="left")
        in_pool = in_cm.__enter__()
        xT = in_pool.tile([128, 8, 1024], F32R)
        nc.sync.dma_start(out=xT[:],
                          in_=inp["xT"].ap().rearrange("(kc p) n -> p kc n", p=128))

        cur = xT  # [128, Cin/128, Nin] transposed activations
        for s, st in enumerate(STAGES):
            cur, in_cm = _stage(nc, tc, psum, ident, inp, stc[s], cur, in_cm,
                                s, st, bo2 if s == 2 else None)

        nc.sync.dma_start(out=out[:], in_=cur[:, 0, :].bitcast(F32))
        in_cm.__exit__(None, None, None)


def _stage(nc, tc, psum, ident, inp, cst, xT, in_cm, s, st, bo_last):
    Cin, d, N = st["Cin"], st["d"], st["N"]
    KC = Cin // 128      # contract chunks of the expand input
    DC = d // 128        # chunks of d
    NT = N // 512        # 512-token tiles
    T = N // 128         # query blocks
    Nh = N // 2
    inv_sqrt_d = float(1.0 / np.sqrt(d))

    my_side = "left" if s % 2 == 0 else "right"   # act{s}/qkv{s} side
    ph_side = "right" if s % 2 == 0 else "left"   # skip/weights/out side

    Exp = mybir.ActivationFunctionType.Exp
    Iden = mybir.ActivationFunctionType.Identity
    Copy = mybir.ActivationFunctionType.Copy
    Add = mybir.AluOpType.add
    Max = mybir.AluOpType.max
    X = mybir.AxisListType.X

    # skipT lives A -> B
    skip_cm = tc.tile_pool(name=f"s{s}_skip", bufs=1, side=ph_side)
    skip_pool = skip_cm.__enter__()
    skipT = skip_pool.tile([128, DC, N], F32R, name=f"skipT{s}")

    # ---- phase A: skipT = interleave(x@We0, x@We1) + enc@Wb + b_skip
    with tc.tile_pool(name=f"s{s}_wA", bufs=1, side=ph_side) as wA, \
         tc.tile_pool(name=f"s{s}_enc", bufs=4, side=ph_side) as encp:
        We0 = wA.tile([128, KC, d], F32R)
        We1 = wA.tile([128, KC, d], F32R)
        Wb = wA.tile([128, DC, d], F32R)
        nc.sync.dma_start(out=We0[:], in_=inp[f"We0_{s}"].ap()
                          .rearrange("(kc p) d -> p kc d", p=128))
        nc.sync.dma_start(out=We1[:], in_=inp[f"We1_{s}"].ap()
                          .rearrange("(kc p) d -> p kc d", p=128))
        nc.sync.dma_start(out=Wb[:], in_=inp[f"Wb_{s}"].ap()
                          .rearrange("(kc p) d -> p kc d", p=128))

        for nt in range(NT):          # 512 output tokens -> 256 even + 256 odd
            h0 = nt * 256             # window into half-token space
            encE = encp.tile([128, DC, 256], F32R, tag="enc")
            encO = encp.tile([128, DC, 256], F32R, tag="enc")
            nc.sync.dma_start(out=encE[:], in_=inp[f"encE{s}"].ap()
                              .rearrange("(kc p) n -> p kc n", p=128)[:, :, h0:h0 + 256])
            nc.sync.dma_start(out=encO[:], in_=inp[f"encO{s}"].ap()
                              .rearrange("(kc p) n -> p kc n", p=128)[:, :, h0:h0 + 256])
            for dch in range(DC):
                pe = psum.tile([128, 256], F32, tag="half")
                po = psum.tile([128, 256], F32, tag="half")
                for kc in range(KC):
                    nc.tensor.matmul(pe[:], We0[:, kc, dch * 128:(dch + 1) * 128],
                                     xT[:, kc, h0:h0 + 256],
                                     start=(kc == 0), stop=False)
                for kc in range(DC):
                    nc.tensor.matmul(pe[:], Wb[:, kc, dch * 128:(dch + 1) * 128],
                                     encE[:, kc, :],
                                     start=False, stop=(kc == DC - 1))
                for kc in range(KC):
                    nc.tensor.matmul(po[:], We1[:, kc, dch * 128:(dch + 1) * 128],
                                     xT[:, kc, h0:h0 + 256],
                                     start=(kc == 0), stop=False)
                for kc in range(DC):
                    nc.tensor.matmul(po[:], Wb[:, kc, dch * 128:(dch + 1) * 128],
                                     encO[:, kc, :],
                                     start=False, stop=(kc == DC - 1))
                nc.scalar.activation(out=skipT[:, dch, 2 * h0:2 * h0 + 512:2],
                                     in_=pe[:], func=Iden,
                                     bias=cst["bskipE"][:, dch:dch + 1])
                nc.scalar.activation(out=skipT[:, dch, 2 * h0 + 1:2 * h0 + 512:2],
                                     in_=po[:], func=Iden,
                                     bias=cst["bskipO"][:, dch:dch + 1])

    in_cm.__exit__(None, None, None)  # frees this stage's input activations

    # QT/KT/V live B -> C
    qkv_cm = tc.tile_pool(name=f"s{s}_qkv", bufs=1, side=my_side)
    qkv_pool = qkv_cm.__enter__()
    QT = qkv_pool.tile([128, DC, N], F32R, name=f"QT{s}")
    KT = qkv_pool.tile([128, DC, N], F32R, name=f"KT{s}")
    V = qkv_pool.tile([128, T + 2, d], F32R, name=f"V{s}")  # T+1: tokens 128..135

    # ---- phase B: QT, KT (transposed), V (natural, -4-shifted grid)
    with tc.tile_pool(name=f"s{s}_wB", bufs=1, side=ph_side) as wB:
        Wq = wB.tile([128, DC, d], F32R)
        Wk = wB.tile([128, DC, d], F32R)
        Wvo = wB.tile([128, DC, d], F32R)
        nc.sync.dma_start(out=Wq[:], in_=inp[f"Wq_{s}"].ap()
                          .rearrange("(kc p) d -> p kc d", p=128))
        nc.sync.dma_start(out=Wk[:], in_=inp[f"Wk_{s}"].ap()
                          .rearrange("(kc p) d -> p kc d", p=128))
        nc.sync.dma_start(out=Wvo[:], in_=inp[f"Wvo_{s}"].ap()
                          .rearrange("(kc p) d -> p kc d", p=128))

        for nt in range(NT):
            n0 = nt * 512
            for dch in range(DC):
                pq = psum.tile([128, 512], F32, tag="mm")
                for kc in range(DC):
                    nc.tensor.matmul(pq[:], Wq[:, kc, dch * 128:(dch + 1) * 128],
                                     skipT[:, kc, n0:n0 + 512],
                                     start=(kc == 0), stop=(kc == DC - 1))
                nc.scalar.activation(out=QT[:, dch, n0:n0 + 512], in_=pq[:],
                                     func=Iden, bias=cst["bq"][:, dch:dch + 1],
                                     scale=inv_sqrt_d)
                pk = psum.tile([128, 512], F32, tag="mm")
                for kc in range(DC):
                    nc.tensor.matmul(pk[:], Wk[:, kc, dch * 128:(dch + 1) * 128],
                                     skipT[:, kc, n0:n0 + 512],
                                     start=(kc == 0), stop=(kc == DC - 1))
                nc.scalar.activation(out=KT[:, dch, n0:n0 + 512], in_=pk[:],
                                     func=Iden, bias=cst["bk"][:, dch:dch + 1])

        for t in range(T + 2):        # V tile t: tokens 128t-4 .. 128t+123
            if t == 0:
                w0, wn = 0, 128
            elif t == T + 1:
                w0, wn = 128, 8       # block-0 tail keys at base partition 0
            else:
                w0 = 128 * t - 4
                wn = min(128, N - w0)
            pv = psum.tile([128, 512], F32, tag="mm")
            for kc in range(DC):
                nc.tensor.matmul(pv[:wn, :d], skipT[:, kc, w0:w0 + wn],
                                 Wvo[:, kc, :],
                                 start=(kc == 0), stop=(kc == DC - 1))
            nc.vector.tensor_copy(V[:wn, t, :], pv[:wn, :d])

    skip_cm.__exit__(None, None, None)

    out_cm = tc.tile_pool(name=f"act{s + 1}", bufs=1, side=ph_side)
    out_pool = out_cm.__enter__()
    outT = out_pool.tile([128, DC, N], F32R, name=f"outT{s}")

    # ---- phase C: per-block attention -> output (transposed, next stage's x)
    with tc.tile_pool(name=f"s{s}_blk", bufs=3, side=my_side) as blk:
        for t in range(T):
            m0 = t * 128
            j0 = 0 if t == 0 else m0 - 4
            W = min(256, N - j0)              # rhs span
            BW = 136 if t < T - 1 else 132    # band width used by softmax
            mask = cst["mask0"] if t == 0 else (
                cst["maskm"] if t < T - 1 else cst["maskl"])

            sc = psum.tile([128, 256], F32, tag="half")
            for kc in range(DC):
                nc.tensor.matmul(sc[:, :W], QT[:, kc, m0:m0 + 128],
                                 KT[:, kc, j0:j0 + W],
                                 start=(kc == 0), stop=(kc == DC - 1))
            rmax = blk.tile([128, 1], F32, tag="rmax")
            nc.vector.tensor_reduce(out=rmax[:], in_=sc[:, :BW], axis=X,
                                    op=Max, negate=True)
            sm = blk.tile([128, 136], F32, tag="sm")
            nc.vector.scalar_tensor_tensor(out=sm[:, :BW], in0=sc[:, :BW],
                                           scalar=rmax[:], in1=mask[:, :BW],
                                           op0=Add, op1=Add)
            pexp = blk.tile([128, 136], F32, tag="pexp")
            rsum = blk.tile([128, 1], F32, tag="rsum")
            nc.scalar.activation(out=pexp[:, :BW], in_=sm[:, :BW], func=Exp,
                                 accum_out=rsum[:])
            rinv = blk.tile([128, 1], F32, tag="rinv")
            nc.vector.reciprocal(out=rinv[:], in_=rsum[:])

            nb = BW - 128                      # rows in the tail transpose
            pt = psum.tile([128, 256], F32, tag="half")
            nc.tensor.transpose(pt[:, 0:128], pexp[:, 0:128], ident[:])
            nc.tensor.transpose(pt[:nb, 128:256], pexp[:, 128:BW], ident[:])
            pt0 = blk.tile([128, 128], F32R, tag="pt0")
            pt1 = blk.tile([8, 128], F32R, tag="pt1")
            nc.vector.tensor_copy(pt0[:], pt[:, 0:128])
            nc.vector.tensor_copy(pt1[:nb, :], pt[:nb, 128:256])

            if t == 0:
                vA, vB = V[:, 0, :], V[0:nb, T + 1, :]
            else:
                vA, vB = V[:, t, :], V[0:nb, t + 1, :]
            av = psum.tile([128, 512], F32, tag="mm")
            nc.tensor.matmul(av[:, :d], pt0[:], vA, start=True, stop=False)
            nc.tensor.matmul(av[:, :d], pt1[:nb, :], vB, start=False, stop=True)
            attn = blk.tile([128, 512], F32, tag="attn")
            nc.scalar.activation(out=attn[:, :d], in_=av[:, :d], func=Copy,
                                 scale=rinv[:])

            tt = psum.tile([128, 512], F32, tag="tt")
            for dch in range(DC):
                nc.tensor.transpose(tt[:, dch * 128:(dch + 1) * 128],
                                    attn[:, dch * 128:(dch + 1) * 128], ident[:])
            if bo_last is not None:
                nc.scalar.activation(out=outT[:, 0, m0:m0 + 128], in_=tt[:, 0:128],
                                     func=Iden, bias=bo_last[:])
            else:
                nc.vector.tensor_copy(
                    outT[:, :, m0:m0 + 128],
                    tt[:, :DC * 128].rearrange("p (dc m) -> p dc m", dc=DC))

    qkv_cm.__exit__(None, None, None)
    return outT, out_cm


# ------------------------------------------------------------------- PJRT run
class _Runner:
    def __init__(self, nc, n_cores=N_CORES):
        import jax
        from jax.sharding import Mesh, PartitionSpec
        from jax.experimental.shard_map import shard_map
        from concourse import bass2jax
        bass2jax.install_neuronx_cc_hook()
        self.jax = jax
        self.n_cores = n_cores
        pname = nc.partition_id_tensor.name if nc.partition_id_tensor else None
        in_names, out_names, out_avals, zero_outs = [], [], [], []
        for alloc in nc.m.functions[0].allocations:
            if not isinstance(alloc, mybir.MemoryLocationSet):
                continue
            name = alloc.memorylocations[0].name
            if alloc.kind == "ExternalInput":
                if name != pname:
                    in_names.append(name)
            elif alloc.kind == "ExternalOutput":
                shape = tuple(alloc.tensor_shape)
                dtype = mybir.dt.np(alloc.dtype)
                out_names.append(name)
                out_avals.append(jax.core.ShapedArray(shape, dtype))
                zero_outs.append(np.zeros(shape, dtype))
        self.in_names, self.out_names = in_names, out_names
        self.out_avals, self.zero_outs = out_avals, zero_outs
        n_params, n_outs = len(in_names), len(out_avals)
        self.n_params = n_params
        all_in = list(in_names) + list(out_names)
        if pname is not None:
            all_in.append(pname)

        def _body(*args):
            operands = list(args)
            if pname is not None:
                operands.append(bass2jax.partition_id_tensor())
            return tuple(bass2jax._bass_exec_p.bind(
                *operands, out_avals=tuple(out_avals), in_names=tuple(all_in),
                out_names=tuple(out_names), lowering_input_output_aliases=(),
                sim_require_finite=True, sim_require_nnan=True, nc=nc))

        devices = jax.devices()[:n_cores]
        mesh = Mesh(np.asarray(devices), ("core",))
        specs = (PartitionSpec("core"),)
        self.fn = jax.jit(
            shard_map(_body, mesh=mesh, in_specs=specs * (n_params + n_outs),
                      out_specs=specs * n_outs, check_rep=False),
            donate_argnums=tuple(range(n_params, n_params + n_outs)),
            keep_unused=True)

    def prep(self, in_maps):
        per = [[np.ascontiguousarray(m[n], dtype=self.out_avals[0].dtype
                                     if False else np.asarray(m[n]).dtype)
                for n in self.in_names] for m in in_maps]
        return [np.concatenate([per[c][i] for c in range(self.n_cores)], axis=0)
                for i in range(self.n_params)]

    def zeros(self):
        return [np.zeros((self.n_cores * z.shape[0], *z.shape[1:]), z.dtype)
                for z in self.zero_outs]

    def run_results(self, in_maps):
        outs = self.fn(*self.prep(in_maps), *self.zeros())
        self.jax.block_until_ready(outs)
        return [{n: np.asarray(outs[i]).reshape(self.n_cores,
                                                *self.out_avals[i].shape)[c]
                 for i, n in enumerate(self.out_names)}
                for c in range(self.n_cores)]

    def time_steady(self, in_maps, iters=8, warmup=2):
        import time
        ci = [self.jax.device_put(a) for a in self.prep(in_maps)]
        for a in ci:
            a.block_until_ready()
        for _ in range(warmup):
            self.jax.block_until_ready(self.fn(*ci, *self.zeros()))
        ts = []
        for _ in range(iters):
            t0 = time.perf_counter()
            self.jax.block_until_ready(self.fn(*ci, *self.zeros()))
            ts.append(time.perf_counter() - t0)
        return min(ts), ts


# -------------------------------------------------------------- host wrapping
def _np(a):
    return np.asarray(a, dtype=np.float32)


def _prep_core_inputs(x_b, encs, params):
    """Per-core (one batch element) input dict. Weight folds are replicated
    but cheap enough to recompute; _prep_shared caches them."""
    d = {"xT": np.ascontiguousarray(x_b.T)}
    return d


def _prep_shared(params, encs_shapes):
    out = {}
    prev_bo = None
    for s, st in enumerate(STAGES):
        Cin, d, N = st["Cin"], st["d"], st["N"]
        p = params["stages"][s]
        W_exp = _np(p["W_exp"]).astype(np.float64)
        W_skip = _np(p["W_skip"]).astype(np.float64)
        b_skip = _np(p["b_skip"]).astype(np.float64)
        Wq, Wk = _np(p["Wq"]), _np(p["Wk"])
        Wv = _np(p["Wv"]).astype(np.float64)
        Wo = _np(p["Wo"]).astype(np.float64)
        bq, bk, bv = (_np(p[n]).astype(np.float64) for n in ("bq", "bk", "bv"))
        bo = _np(p["bo"]).astype(np.float64)
        rel = _np(p["rel_bias"]).astype(np.float64)

        Wtop, Wbot = W_skip[:d], W_skip[d:]
        We0 = (W_exp[:, :d] @ Wtop).astype(np.float32)
        We1 = (W_exp[:, d:] @ Wtop).astype(np.float32)
        out[f"We0_{s}"], out[f"We1_{s}"] = We0, We1
        out[f"Wb_{s}"] = Wbot.astype(np.float32)
        out[f"Wq_{s}"], out[f"Wk_{s}"] = Wq, Wk
        out[f"Wvo_{s}"] = (Wv @ Wo).astype(np.float32)

        bskipE = b_skip.copy()
        bskipO = b_skip.copy()
        if prev_bo is not None:
            bskipE = bskipE + prev_bo @ (W_exp[:, :d] @ Wtop)
            bskipO = bskipO + prev_bo @ (W_exp[:, d:] @ Wtop)
        bo_eff = bv @ Wo + bo
        prev_bo = bo_eff

        def col(v):
            return np.ascontiguousarray(
                v.astype(np.float32).reshape(d // 128, 128).T)

        out[f"bskipE_{s}"], out[f"bskipO_{s}"] = col(bskipE), col(bskipO)
        out[f"bq_{s}"] = col(bq / np.sqrt(d))
        out[f"bk_{s}"] = col(bk)

        T = N // 128
        for nm, t in (("mask0", 0), ("maskm", 1), ("maskl", T - 1)):
            m0 = t * 128
            j0 = 0 if t == 0 else m0 - 4
            BW = 136 if t < T - 1 else 132
            acc = np.zeros((128, BW), dtype=np.float64)
            for m in range(128):
                n = m0 + m
                for k in range(K_NEIGH):
                    key = min(max(n + k - 4, 0), N - 1)
                    acc[m, key - j0] += np.exp(rel[n % K_NEIGH, k] / np.sqrt(d))
            mask = np.full((128, BW), -30000.0)
            nz = acc > 0
            mask[nz] = np.log(acc[nz])
            out[f"{nm}_{s}"] = mask.astype(np.float32)
        if s == 2:
            out["bo_2"] = np.ascontiguousarray(
                bo_eff.astype(np.float32).reshape(1, 128).T)
    return out


_BUILT = {}


def _get_runner(n_iters=1):
    key = n_iters
    if key not in _BUILT:
        nc = build_decoder(n_iters)
        _BUILT[key] = _Runner(nc)
    return _BUILT[key]


def _make_in_maps(x, enc0, enc1, enc2, params):
    x = _np(x)
    encs = [_np(enc0), _np(enc1), _np(enc2)]
    shared = _prep_shared(params, None)
    for s in range(3):
        N = STAGES[s]["N"]
    in_maps = []
    for b in range(N_CORES):
        m = dict(shared)
        m["xT"] = np.ascontiguousarray(x[b].T)
        for s in range(3):
            eT = encs[s][b].T  # [d, N]
            m[f"encE{s}"] = np.ascontiguousarray(eT[:, 0::2])
            m[f"encO{s}"] = np.ascontiguousarray(eT[:, 1::2])
        in_maps.append(m)
    return in_maps


def kernel(x, enc0, enc1, enc2, params):
    runner = _get_runner(1)
    in_maps = _make_in_maps(x, enc0, enc1, enc2, params)
    res = runner.run_results(in_maps)
    out = np.stack([res[b]["out"].T for b in range(N_CORES)], axis=0)
    return out.astype(np.float32)
